# revision 25
# baseline (speedup 1.0000x reference)
"""GCN layer (SpMM): out[r] = sum_{e: row(e)=r} val[e] * embeds[col(e)]
for N=100000 nodes, d=128, E=3200000 edges, distributed over 8 NeuronCores.

Sharding: 1D row partition — core k owns destination rows [k*12500, (k+1)*12500);
the embedding table is replicated. Per core the edges are bucketed by
(128-row output window, 25000-row column chunk); each bucket is padded to a
common slot budget so one SPMD program serves all cores.

Key design points (see git history for the f32 baseline this evolved from):
  - embeds cast to bf16 on host: fp32 matmuls run at 1/4 rate on the PE
    (2 half-speed passes), bf16 at full rate; and each gather descriptor
    moves 256B instead of 512B.
  - buckets are sorted by column index so the random HBM reads of one
    bucket walk addresses monotonically (row-buffer locality).
  - buckets are padded with idx=-1 and the true count is passed through
    num_idxs_reg (loaded from SBUF at runtime), so the SWDGE skips the
    padding descriptors entirely instead of gathering dummy rows.
  - `group` consecutive windows share one dma_gather per chunk, amortizing
    the ~1us fixed SWDGE cost per call; interior windows pad with idx 0
    (gathered but weighted 0), only the last window's padding is skipped.

Device pipeline per window group:
  - NCH dma_gathers (one per column chunk, int16 chunk-relative indices)
    pull the 256B bf16 embedding rows for the group's edges into SBUF.
  - per 128-edge subtile, one fused DVE tensor_scalar builds the
    val-weighted one-hot S[e, r] = val[e] * (row_rel[e] == r).
  - TensorE matmuls S^T @ G accumulate each window's [128,128] block in
    PSUM; PSUM -> SBUF -> DRAM.
"""

import os
import sys

import numpy as np

for _p in ("/opt/trn_rl_repo", "/root/problem"):
    if _p not in sys.path:
        sys.path.insert(0, _p)

os.environ.setdefault("NEURON_RT_RESET_CORES", "1")

import ml_dtypes

BF16 = np.dtype(ml_dtypes.bfloat16)

N_NODES = 100000
D = 128
N_CORES = 8
B = N_NODES // N_CORES          # 12500 destination rows per core
WIN = 128                       # output window rows (= PSUM partition dim)
NW = 100                        # windows per core (2 windows of slack so the
                                # balancer can hold every bucket <= 1024)
B_PAD = NW * WIN                # 12800 padded rows per core
NCH = 4                         # column chunks (int16 index range)
CH = N_NODES // NCH             # 25000 rows per chunk

_cache = {}

# compute-path dtype, bucket sorting, padding-skip, and gather grouping are
# switchable so the perf harness can A/B them; defaults ship.
CFG = dict(bf16=True, sort=True, skip=True, group=2, psum_iota=True,
           act_mod=8, iota_mode=None, balance=True)


def _build(budget, repeat=1, bf16=None, skip=None, group=None,
           psum_iota=None, single_packet=False, one_queue=False,
           dma_scratch=16384, act_mod=None, gather_only=False,
           compute_only=False, no_ts=False, no_mm=False, iota_mode=None,
           g_bufs=None):
    """Build + schedule the SPMD bass program for a per-(window,chunk) slot
    budget (multiple of 128). Returns the compiled Bacc module.

    repeat > 1 wraps the compute body in an on-device For_i loop — used only
    by the perf harness to amortize dispatch overhead when measuring."""
    import contextlib

    import concourse.mybir as mybir
    import concourse.tile as tile
    from concourse import bacc

    if bf16 is None:
        bf16 = CFG["bf16"]
    if skip is None:
        skip = CFG["skip"]
    if group is None:
        group = CFG["group"]
    if psum_iota is None:
        psum_iota = CFG["psum_iota"]
    if act_mod is None:
        act_mod = CFG["act_mod"]
    if iota_mode is None:
        iota_mode = CFG.get("iota_mode")
    # iota_mode: None -> legacy f32 iota (PSUM or SBUF per psum_iota);
    # 'bf16_4x' -> bf16 SBUF iota: DVE picks a 2-port packed mode, which
    # locks GPSIMD out of the shared SBUF port pair and starves SWDGE
    # descriptor gen (measured: gathers fully serialize against S-builds);
    # 'bf16_2x' -> bf16 SBUF iota, [128,129] S (odd innermost) — measured
    # identical to bf16_4x, both blocking;
    # 'psum_bf16' -> bf16 iota in its own PSUM bank: in0 arrives via the
    # PSUM port, per-partition scalars via the dedicated rd0, S via the
    # dedicated write port -> 2x_1P at 120+FD/2 cycles with ZERO shared-pair
    # usage. Non-blocking and 1.35x faster than the legacy f32 PSUM read.
    SW = 129 if iota_mode == "bf16_2x" else 128
    GRP = group
    NP = NW // GRP
    assert NP * GRP == NW

    nsub_ch = budget // 128          # subtiles per (window, chunk) segment
    nsub = NCH * nsub_ch             # subtiles per window
    gb = GRP * budget                # slots per gather call
    gidx_cols = gb // 16             # idx16 columns per (group, chunk)
    if g_bufs is None:
        g_bufs = 6 if GRP == 1 else (3 if GRP == 2 else 2)

    nc = bacc.Bacc("TRN2", target_bir_lowering=False, debug=False,
                   num_devices=N_CORES, num_swdge_queues=4,
                   dynamic_dma_scratch_size=dma_scratch)
    gdt = mybir.dt.bfloat16 if bf16 else mybir.dt.float32
    embeds = nc.dram_tensor("embeds", [N_NODES, D], gdt,
                            kind="ExternalInput")
    idx16 = nc.dram_tensor("idx16", [128, NP * NCH * gidx_cols],
                           mybir.dt.int16, kind="ExternalInput")
    rows_rel = nc.dram_tensor("rows_rel", [128, NW * nsub], mybir.dt.float32,
                              kind="ExternalInput")
    vals = nc.dram_tensor("vals", [128, NW * nsub], mybir.dt.float32,
                          kind="ExternalInput")
    neg_rows = nc.dram_tensor("neg_rows", [128, NW * nsub], mybir.dt.float32,
                              kind="ExternalInput")
    neg_vals = nc.dram_tensor("neg_vals", [128, NW * nsub], mybir.dt.float32,
                              kind="ExternalInput")
    counts = nc.dram_tensor("counts", [1, NP * NCH], mybir.dt.int32,
                            kind="ExternalInput") if skip else None
    out = nc.dram_tensor("out", [B_PAD, D], mybir.dt.float32,
                         kind="ExternalOutput")

    with tile.TileContext(nc) as tc:
        with (
            tc.tile_pool(name="const", bufs=1) as const_pool,
            tc.tile_pool(name="gather", bufs=g_bufs) as g_pool,
            tc.tile_pool(name="s", bufs=12) as s_pool,
            tc.tile_pool(name="sa", bufs=6) as sa_pool,
            tc.tile_pool(name="t", bufs=6) as t_pool,
            tc.tile_pool(name="o", bufs=4) as o_pool,
            tc.tile_pool(name="psum", bufs=7, space="PSUM") as psum_pool,
            tc.tile_pool(name="iotap", bufs=1, space="PSUM") as iota_pool,
        ):
            iota_i = const_pool.tile([128, SW], mybir.dt.int32)
            nc.gpsimd.iota(iota_i[:], pattern=[[1, SW]], base=0,
                           channel_multiplier=0)
            if iota_mode == "psum_bf16":
                # bf16 iota in its own full PSUM bank (1024 bf16 = 2KB per
                # partition) so it never shares a bank with a PE accumulator.
                iota_bank = iota_pool.tile([128, 1024], mybir.dt.bfloat16,
                                           space="PSUM")
                iota_f = iota_bank[:, 0:128]
            elif iota_mode is not None:
                # bf16 SBUF iota: integer values 0..128 are exact in bf16 and
                # unlock the DVE 16-bit packed perf modes for the S-build.
                iota_f = const_pool.tile([128, SW], mybir.dt.bfloat16)
            elif psum_iota:
                # legacy: iota lives in PSUM so the f32 tensor_scalar stays in
                # 1x perf mode and never takes the shared SBUF port pair that
                # SWDGE descriptor generation (gpsimd) needs. Pad to a full
                # 2KB-per-partition PSUM bank so the iota never shares a bank
                # with a PE-written accumulator.
                iota_bank = iota_pool.tile([128, 512], mybir.dt.float32,
                                           space="PSUM")
                iota_f = iota_bank[:, 0:128]
            else:
                iota_f = const_pool.tile([128, 128], mybir.dt.float32)
            nc.vector.tensor_copy(out=iota_f[:], in_=iota_i[:])

            idx_all = const_pool.tile([128, NP * NCH * gidx_cols],
                                      mybir.dt.int16)
            nc.sync.dma_start(out=idx_all[:], in_=idx16[:])
            rows_all = const_pool.tile([128, NW * nsub], mybir.dt.float32)
            nc.sync.dma_start(out=rows_all[:], in_=rows_rel[:])
            vals_all = const_pool.tile([128, NW * nsub], mybir.dt.float32)
            nc.sync.dma_start(out=vals_all[:], in_=vals[:])
            nrows_all = const_pool.tile([128, NW * nsub], mybir.dt.float32)
            nc.sync.dma_start(out=nrows_all[:], in_=neg_rows[:])
            nvals_all = const_pool.tile([128, NW * nsub], mybir.dt.float32)
            nc.sync.dma_start(out=nvals_all[:], in_=neg_vals[:])
            # ACT-side S-build needs a plain SBUF f32 iota even when the DVE
            # side reads its iota from PSUM or as bf16
            iota_s = const_pool.tile([128, 128], mybir.dt.float32)
            nc.vector.tensor_copy(out=iota_s[:], in_=iota_i[:, 0:128])
            if skip:
                cnt_all = const_pool.tile([1, NP * NCH], mybir.dt.int32)
                nc.sync.dma_start(out=cnt_all[:], in_=counts[:])
                # Pre-zero the gather ring so slots the SWDGE skips (trailing
                # -1 indices) never expose NaN garbage to the 0*x matmul
                # terms.
                if not compute_only:
                    for _ in range(g_bufs):
                        gz = g_pool.tile([128, NCH * gb], gdt, tag="G")
                        nc.vector.memset(gz[:], 0)
                cnt_regs = [
                    nc.gpsimd.alloc_register(f"cnt{c}") for c in range(NCH)
                ]
            if compute_only:
                g_fixed = const_pool.tile([128, NCH * gb], gdt)
                nc.vector.memset(g_fixed[:], 0)
            if no_ts:
                s_fixed = const_pool.tile([128, 128], gdt)
                nc.vector.memset(s_fixed[:], 0)

            loop = tc.For_i(0, repeat, 1) if repeat > 1 else contextlib.nullcontext()
            with loop:
                for p in range(NP):
                    G = g_fixed if compute_only else g_pool.tile(
                        [128, NCH * gb], gdt, tag="G")
                    if not compute_only:
                        for c in range(NCH):
                            seg = G[:, c * gb : (c + 1) * gb]
                            ioff = (p * NCH + c) * gidx_cols
                            if skip:
                                cnt = cnt_regs[c]
                                nc.gpsimd.reg_load(
                                    cnt,
                                    cnt_all[0:1, p * NCH + c : p * NCH + c + 1])
                            else:
                                cnt = gb
                            nc.gpsimd.dma_gather(
                                out_ap=seg.rearrange("p (j d) -> p j d", d=128),
                                in_ap=embeds[c * CH : (c + 1) * CH, :],
                                idxs_ap=idx_all[:, ioff : ioff + gidx_cols],
                                num_idxs=gb,
                                num_idxs_reg=cnt,
                                elem_size=D,
                                single_packet=single_packet,
                                queue_num=0 if one_queue else c,
                            )
                    if gather_only:
                        continue
                    for i in range(GRP):
                        w = p * GRP + i
                        acc = None if no_mm else psum_pool.tile(
                            [128, 128], mybir.dt.float32, space="PSUM")
                        k = 0
                        for c in range(NCH):
                            for j in range(nsub_ch):
                                scol = ((p * NCH + c) * GRP + i) * nsub_ch + j
                                if no_ts:
                                    S = s_fixed
                                elif act_mod and (
                                    (k % 5 in (2, 4)) if act_mod == 5
                                    else (k % 8 in (2, 5, 7)) if act_mod == 8
                                    else (k % 12 in (1, 3, 6, 8, 11))
                                    if act_mod == 12
                                    else k % act_mod == act_mod - 1
                                ):
                                    # ACT lane: T = |iota - row|;
                                    # S = relu(val - val*T) = val iff T==0
                                    T = t_pool.tile([128, 128],
                                                    mybir.dt.float32, tag="T")
                                    nc.scalar.activation(
                                        out=T[:], in_=iota_s[:],
                                        func=mybir.ActivationFunctionType.Abs,
                                        bias=nrows_all[:, scol : scol + 1],
                                    )
                                    S = sa_pool.tile([128, 128], gdt, tag="SA")
                                    nc.scalar.activation(
                                        out=S[:], in_=T[:],
                                        func=mybir.ActivationFunctionType.Relu,
                                        bias=vals_all[:, scol : scol + 1],
                                        scale=nvals_all[:, scol : scol + 1],
                                    )
                                else:
                                    S = s_pool.tile([128, SW], gdt, tag="S")
                                    nc.vector.tensor_scalar(
                                        out=S[:],
                                        in0=iota_f[:],
                                        scalar1=rows_all[:, scol : scol + 1],
                                        scalar2=vals_all[:, scol : scol + 1],
                                        op0=mybir.AluOpType.is_equal,
                                        op1=mybir.AluOpType.mult,
                                    )
                                gcol = (c * GRP + i) * budget + j * 128
                                if not no_mm:
                                    nc.tensor.matmul(
                                        out=acc[:],
                                        lhsT=S[:, 0:128],
                                        rhs=G[:, gcol : gcol + 128],
                                        start=(k == 0),
                                        stop=(k == nsub - 1),
                                    )
                                k += 1
                        if not no_mm:
                            o = o_pool.tile([128, 128], mybir.dt.float32)
                            nc.scalar.copy(out=o[:], in_=acc[:])
                            nc.sync.dma_start(
                                out=out[w * 128 : (w + 1) * 128, :], in_=o[:])

    nc.compile()
    return nc


def _balance(row_local, ch, core, limit=1024):
    """Assign each core's local rows to windows so per-(window, chunk) edge
    counts are near-uniform (greedy 4-dim vector bin packing, largest rows
    first, then swap-repair toward max <= limit). Returns (w, pos) per
    edge-row mapping arrays [N_CORES, B]."""
    w_of = np.empty((N_CORES, B), np.int64)
    pos_of = np.empty((N_CORES, B), np.int64)
    for k in range(N_CORES):
        m = core == k
        deg = np.zeros((B, NCH), np.int64)
        np.add.at(deg, (row_local[m], ch[m]), 1)
        order = np.argsort(-deg.sum(1), kind="stable")
        loads = np.zeros((NW, NCH), np.int64)
        cnt = np.zeros(NW, np.int64)
        wk = np.empty(B, np.int64)
        big = 1 << 50
        for r in order:
            cand = np.max(loads + deg[r], axis=1)
            cand[cnt >= WIN] = big
            wsel = int(np.argmin(cand))
            wk[r] = wsel
            loads[wsel] += deg[r]
            cnt[wsel] += 1
        # swap-repair: shave (window, chunk) cells above `limit`. Each accepted
        # swap leaves BOTH windows fully <= limit (feasibility is checked on
        # all 4 chunks), so over-limit cells strictly decrease -> terminates.
        stuck: set = set()
        for _ in range(800):
            over = np.argwhere(loads > limit)
            over = [t for t in map(tuple, over) if t not in stuck]
            if not over:
                break
            w1, c1 = max(over, key=lambda t: loads[t])
            rows_w1 = np.nonzero(wk == w1)[0]
            done = False
            for r1 in rows_w1[np.argsort(-deg[rows_w1, c1])][:32]:
                d1 = deg[r1]
                free_w = np.nonzero((cnt < WIN)
                                    & (np.arange(NW) != w1))[0]
                if len(free_w) and (loads[w1] - d1 <= limit).all():
                    okm = free_w[(loads[free_w] + d1 <= limit).all(1)]
                    if len(okm):
                        w2 = okm[np.argmin(loads[okm, c1])]
                        wk[r1] = w2
                        loads[w1] -= d1
                        loads[w2] += d1
                        cnt[w1] -= 1
                        cnt[w2] += 1
                        done = True
                        break
                for w2 in np.argsort(loads[:, c1]):
                    if w2 == w1:
                        continue
                    rows_w2 = np.nonzero(wk == w2)[0]
                    degs2 = deg[rows_w2]
                    ok = (((loads[w1] - d1)[None, :] + degs2 <= limit).all(1)
                          & ((loads[w2] + d1)[None, :] - degs2 <= limit).all(1))
                    idx2 = np.nonzero(ok)[0]
                    if len(idx2):
                        r2 = rows_w2[idx2[np.argmin(degs2[idx2, c1])]]
                        d2 = deg[r2]
                        wk[r1], wk[r2] = w2, w1
                        loads[w1] += d2 - d1
                        loads[w2] += d1 - d2
                        done = True
                        break
                if done:
                    break
            if not done:
                stuck.add((w1, c1))
        w_of[k] = wk
        for wsel in range(NW):
            rows_in = np.nonzero(wk == wsel)[0]
            pos_of[k, rows_in] = np.arange(len(rows_in))
    return w_of, pos_of


def _prep(edge_index, edge_vals, sort=None, skip=None, group=None,
          balance=None):
    """Bucket + sort + pad edges; returns (budget, per-core input dicts,
    per-core output row permutation out_perm [N_CORES, B] s.t.
    out[orig_row] = device_out[out_perm[orig_row]])."""
    if sort is None:
        sort = CFG["sort"]
    if skip is None:
        skip = CFG["skip"]
    if group is None:
        group = CFG["group"]
    if balance is None:
        balance = CFG.get("balance", True)
    GRP = group
    NP = NW // GRP

    rows = np.asarray(edge_index[0], dtype=np.int64)
    cols = np.asarray(edge_index[1], dtype=np.int64)
    vals = np.asarray(edge_vals, dtype=np.float32)
    E = rows.shape[0]

    core = rows // B
    row_local = rows - core * B
    ch = cols // CH
    col_rel = (cols - ch * CH).astype(np.int16)
    if balance:
        w_of, pos_of = _balance(row_local, ch, core)
        w = w_of[core, row_local]
        row_rel = pos_of[core, row_local].astype(np.float32)
        out_perm = w_of * WIN + pos_of          # [N_CORES, B]
    else:
        w = row_local // WIN
        row_rel = (row_local - w * WIN).astype(np.float32)
        out_perm = np.broadcast_to(np.arange(B, dtype=np.int64),
                                   (N_CORES, B))

    # slot-space order: (core, group, chunk, window-in-group, pos)
    p = w // GRP
    i = w - p * GRP
    bucket = (((core * NP + p) * NCH + ch) * GRP + i).astype(np.int64)
    n_buckets = N_CORES * NW * NCH
    counts = np.bincount(bucket, minlength=n_buckets)
    budget = int(-(-counts.max() // 128) * 128)

    # bucket-major, column-sorted within bucket: monotone HBM addresses
    order = np.lexsort((cols, bucket)) if sort else np.argsort(bucket, kind="stable")
    starts = np.zeros(n_buckets, dtype=np.int64)
    np.cumsum(counts[:-1], out=starts[1:])
    pos = np.arange(E, dtype=np.int64) - starts[bucket[order]]

    bo = bucket[order]
    slot = bo * budget + pos            # global slot id across all cores

    n_slots = n_buckets * budget
    # interior windows of a group pad with 0 (gathered, weight 0); the last
    # window's padding is -1 (descriptor-skipped) when skip is on
    if skip:
        idx_lin = np.zeros(n_slots, dtype=np.int16)
        tail = (np.arange(n_buckets, dtype=np.int64) % GRP) == GRP - 1
        idx_lin[np.repeat(tail, budget)] = -1
    else:
        idx_lin = np.zeros(n_slots, dtype=np.int16)
    rows_lin = np.zeros(n_slots, dtype=np.float32)
    vals_lin = np.zeros(n_slots, dtype=np.float32)
    idx_lin[slot] = col_rel[order]
    rows_lin[slot] = row_rel[order]
    vals_lin[slot] = vals[order]

    # per-(group, chunk) gather count: interior windows fully gathered, the
    # last window only its real edges
    cg = counts.reshape(-1, GRP)  # [cores*NP*NCH, GRP]
    call_counts = (GRP - 1) * budget + cg[:, GRP - 1]
    if skip and GRP == 1:
        # an all-negative index list is invalid; give empty buckets one
        # dummy slot (gather row 0, weight 0)
        empty = call_counts == 0
        if empty.any():
            eb = np.nonzero(empty)[0]
            idx_lin[eb * budget] = 0
        call_counts = np.maximum(call_counts, 1)
    call_counts = call_counts.astype(np.int32)

    in_maps = []
    per_core = NW * NCH * budget
    n_calls = NP * NCH
    for k in range(N_CORES):
        lin = slice(k * per_core, (k + 1) * per_core)
        # idx16: slot q of a call -> (partition q%16, col q//16), tiled x8
        a = idx_lin[lin].reshape(n_calls, GRP * budget // 16, 16)
        idx16 = np.ascontiguousarray(a.transpose(2, 0, 1).reshape(16, -1))
        idx16 = np.tile(idx16, (8, 1))
        # rows/vals: slot s -> (partition s%128, col s//128)
        rows_t = np.ascontiguousarray(rows_lin[lin].reshape(-1, 128).T)
        vals_t = np.ascontiguousarray(vals_lin[lin].reshape(-1, 128).T)
        m = {"idx16": idx16, "rows_rel": rows_t, "vals": vals_t,
             "neg_rows": -rows_t, "neg_vals": -vals_t}
        if skip:
            m["counts"] = call_counts[k * n_calls : (k + 1) * n_calls].reshape(
                1, -1)
        in_maps.append(m)
    return budget, in_maps, out_perm


def kernel(embeds, edge_index, edge_vals):
    from concourse.bass_utils import run_bass_kernel_spmd

    embeds = np.ascontiguousarray(np.asarray(embeds, dtype=np.float32))
    if CFG["bf16"]:
        embeds = embeds.astype(BF16)
    budget, in_maps, out_perm = _prep(edge_index, edge_vals)
    for m in in_maps:
        m["embeds"] = embeds

    if budget not in _cache:
        _cache[budget] = _build(budget)
    nc = _cache[budget]

    res = run_bass_kernel_spmd(nc, in_maps, core_ids=list(range(N_CORES)))
    out = np.empty((N_NODES, D), dtype=np.float32)
    for k in range(N_CORES):
        out[k * B : (k + 1) * B] = res.results[k]["out"][out_perm[k]]
    return out



# revision 26
# speedup vs baseline: 1.0453x; 1.0453x over previous
"""GCN layer (SpMM): out[r] = sum_{e: row(e)=r} val[e] * embeds[col(e)]
for N=100000 nodes, d=128, E=3200000 edges, distributed over 8 NeuronCores.

Sharding: 1D row partition — core k owns destination rows [k*12500, (k+1)*12500);
the embedding table is replicated. Per core the edges are bucketed by
(128-row output window, 25000-row column chunk); each bucket is padded to a
common slot budget so one SPMD program serves all cores.

Key design points (see git history for the f32 baseline this evolved from):
  - embeds cast to bf16 on host: fp32 matmuls run at 1/4 rate on the PE
    (2 half-speed passes), bf16 at full rate; and each gather descriptor
    moves 256B instead of 512B.
  - buckets are sorted by column index so the random HBM reads of one
    bucket walk addresses monotonically (row-buffer locality).
  - buckets are padded with idx=-1 and the true count is passed through
    num_idxs_reg (loaded from SBUF at runtime), so the SWDGE skips the
    padding descriptors entirely instead of gathering dummy rows.
  - `group` consecutive windows share one dma_gather per chunk, amortizing
    the ~1us fixed SWDGE cost per call; interior windows pad with idx 0
    (gathered but weighted 0), only the last window's padding is skipped.

Device pipeline per window group:
  - NCH dma_gathers (one per column chunk, int16 chunk-relative indices)
    pull the 256B bf16 embedding rows for the group's edges into SBUF.
  - per 128-edge subtile, one fused DVE tensor_scalar builds the
    val-weighted one-hot S[e, r] = val[e] * (row_rel[e] == r).
  - TensorE matmuls S^T @ G accumulate each window's [128,128] block in
    PSUM; PSUM -> SBUF -> DRAM.
"""

import os
import sys

import numpy as np

for _p in ("/opt/trn_rl_repo", "/root/problem"):
    if _p not in sys.path:
        sys.path.insert(0, _p)

os.environ.setdefault("NEURON_RT_RESET_CORES", "1")

import ml_dtypes

BF16 = np.dtype(ml_dtypes.bfloat16)

N_NODES = 100000
D = 128
N_CORES = 8
B = N_NODES // N_CORES          # 12500 destination rows per core
WIN = 128                       # output window rows (= PSUM partition dim)
NW = 100                        # windows per core (2 windows of slack so the
                                # balancer can hold every bucket <= 1024)
B_PAD = NW * WIN                # 12800 padded rows per core
NCH = 4                         # column chunks (int16 index range)
CH = N_NODES // NCH             # 25000 rows per chunk

_cache = {}

# compute-path dtype, bucket sorting, padding-skip, and gather grouping are
# switchable so the perf harness can A/B them; defaults ship.
CFG = dict(bf16=True, sort=True, skip=True, group=2, psum_iota=True,
           act_mod=12, iota_mode=None, balance=True)


def _build(budget, repeat=1, bf16=None, skip=None, group=None,
           psum_iota=None, single_packet=False, one_queue=False,
           dma_scratch=16384, act_mod=None, gather_only=False,
           compute_only=False, no_ts=False, no_mm=False, iota_mode=None,
           g_bufs=None):
    """Build + schedule the SPMD bass program for a per-(window,chunk) slot
    budget (multiple of 128). Returns the compiled Bacc module.

    repeat > 1 wraps the compute body in an on-device For_i loop — used only
    by the perf harness to amortize dispatch overhead when measuring."""
    import contextlib

    import concourse.mybir as mybir
    import concourse.tile as tile
    from concourse import bacc

    if bf16 is None:
        bf16 = CFG["bf16"]
    if skip is None:
        skip = CFG["skip"]
    if group is None:
        group = CFG["group"]
    if psum_iota is None:
        psum_iota = CFG["psum_iota"]
    if act_mod is None:
        act_mod = CFG["act_mod"]
    if iota_mode is None:
        iota_mode = CFG.get("iota_mode")
    # iota_mode: None -> legacy f32 iota (PSUM or SBUF per psum_iota);
    # 'bf16_4x' -> bf16 SBUF iota: DVE picks a 2-port packed mode, which
    # locks GPSIMD out of the shared SBUF port pair and starves SWDGE
    # descriptor gen (measured: gathers fully serialize against S-builds);
    # 'bf16_2x' -> bf16 SBUF iota, [128,129] S (odd innermost) — measured
    # identical to bf16_4x, both blocking;
    # 'psum_bf16' -> bf16 iota in its own PSUM bank: in0 arrives via the
    # PSUM port, per-partition scalars via the dedicated rd0, S via the
    # dedicated write port -> 2x_1P at 120+FD/2 cycles with ZERO shared-pair
    # usage. Non-blocking and 1.35x faster than the legacy f32 PSUM read.
    SW = 129 if iota_mode == "bf16_2x" else 128
    GRP = group
    NP = NW // GRP
    assert NP * GRP == NW

    nsub_ch = budget // 128          # subtiles per (window, chunk) segment
    nsub = NCH * nsub_ch             # subtiles per window
    gb = GRP * budget                # slots per gather call
    gidx_cols = gb // 16             # idx16 columns per (group, chunk)
    if g_bufs is None:
        g_bufs = 6 if GRP == 1 else (3 if GRP == 2 else 2)

    nc = bacc.Bacc("TRN2", target_bir_lowering=False, debug=False,
                   num_devices=N_CORES, num_swdge_queues=4,
                   dynamic_dma_scratch_size=dma_scratch)
    gdt = mybir.dt.bfloat16 if bf16 else mybir.dt.float32
    embeds = nc.dram_tensor("embeds", [N_NODES, D], gdt,
                            kind="ExternalInput")
    idx16 = nc.dram_tensor("idx16", [128, NP * NCH * gidx_cols],
                           mybir.dt.int16, kind="ExternalInput")
    rows_rel = nc.dram_tensor("rows_rel", [128, NW * nsub], mybir.dt.float32,
                              kind="ExternalInput")
    vals = nc.dram_tensor("vals", [128, NW * nsub], mybir.dt.float32,
                          kind="ExternalInput")
    neg_rows = nc.dram_tensor("neg_rows", [128, NW * nsub], mybir.dt.float32,
                              kind="ExternalInput")
    neg_vals = nc.dram_tensor("neg_vals", [128, NW * nsub], mybir.dt.float32,
                              kind="ExternalInput")
    counts = nc.dram_tensor("counts", [1, NP * NCH], mybir.dt.int32,
                            kind="ExternalInput") if skip else None
    out = nc.dram_tensor("out", [B_PAD, D], mybir.dt.float32,
                         kind="ExternalOutput")

    with tile.TileContext(nc) as tc:
        with (
            tc.tile_pool(name="const", bufs=1) as const_pool,
            tc.tile_pool(name="gather", bufs=g_bufs) as g_pool,
            tc.tile_pool(name="s", bufs=12) as s_pool,
            tc.tile_pool(name="sa", bufs=6) as sa_pool,
            tc.tile_pool(name="t", bufs=6) as t_pool,
            tc.tile_pool(name="o", bufs=4) as o_pool,
            tc.tile_pool(name="psum", bufs=7, space="PSUM") as psum_pool,
            tc.tile_pool(name="iotap", bufs=1, space="PSUM") as iota_pool,
        ):
            iota_i = const_pool.tile([128, SW], mybir.dt.int32)
            nc.gpsimd.iota(iota_i[:], pattern=[[1, SW]], base=0,
                           channel_multiplier=0)
            if iota_mode == "psum_bf16":
                # bf16 iota in its own full PSUM bank (1024 bf16 = 2KB per
                # partition) so it never shares a bank with a PE accumulator.
                iota_bank = iota_pool.tile([128, 1024], mybir.dt.bfloat16,
                                           space="PSUM")
                iota_f = iota_bank[:, 0:128]
            elif iota_mode is not None:
                # bf16 SBUF iota: integer values 0..128 are exact in bf16 and
                # unlock the DVE 16-bit packed perf modes for the S-build.
                iota_f = const_pool.tile([128, SW], mybir.dt.bfloat16)
            elif psum_iota:
                # legacy: iota lives in PSUM so the f32 tensor_scalar stays in
                # 1x perf mode and never takes the shared SBUF port pair that
                # SWDGE descriptor generation (gpsimd) needs. Pad to a full
                # 2KB-per-partition PSUM bank so the iota never shares a bank
                # with a PE-written accumulator.
                iota_bank = iota_pool.tile([128, 512], mybir.dt.float32,
                                           space="PSUM")
                iota_f = iota_bank[:, 0:128]
            else:
                iota_f = const_pool.tile([128, 128], mybir.dt.float32)
            nc.vector.tensor_copy(out=iota_f[:], in_=iota_i[:])

            idx_all = const_pool.tile([128, NP * NCH * gidx_cols],
                                      mybir.dt.int16)
            nc.sync.dma_start(out=idx_all[:], in_=idx16[:])
            rows_all = const_pool.tile([128, NW * nsub], mybir.dt.float32)
            nc.sync.dma_start(out=rows_all[:], in_=rows_rel[:])
            vals_all = const_pool.tile([128, NW * nsub], mybir.dt.float32)
            nc.sync.dma_start(out=vals_all[:], in_=vals[:])
            nrows_all = const_pool.tile([128, NW * nsub], mybir.dt.float32)
            nc.sync.dma_start(out=nrows_all[:], in_=neg_rows[:])
            nvals_all = const_pool.tile([128, NW * nsub], mybir.dt.float32)
            nc.sync.dma_start(out=nvals_all[:], in_=neg_vals[:])
            # ACT-side S-build needs a plain SBUF f32 iota even when the DVE
            # side reads its iota from PSUM or as bf16
            iota_s = const_pool.tile([128, 128], mybir.dt.float32)
            nc.vector.tensor_copy(out=iota_s[:], in_=iota_i[:, 0:128])
            if skip:
                cnt_all = const_pool.tile([1, NP * NCH], mybir.dt.int32)
                nc.sync.dma_start(out=cnt_all[:], in_=counts[:])
                # Pre-zero the gather ring so slots the SWDGE skips (trailing
                # -1 indices) never expose NaN garbage to the 0*x matmul
                # terms.
                if not compute_only:
                    for _ in range(g_bufs):
                        gz = g_pool.tile([128, NCH * gb], gdt, tag="G")
                        nc.vector.memset(gz[:], 0)
                cnt_regs = [
                    nc.gpsimd.alloc_register(f"cnt{c}") for c in range(NCH)
                ]
            if compute_only:
                g_fixed = const_pool.tile([128, NCH * gb], gdt)
                nc.vector.memset(g_fixed[:], 0)
            if no_ts:
                s_fixed = const_pool.tile([128, 128], gdt)
                nc.vector.memset(s_fixed[:], 0)

            loop = tc.For_i(0, repeat, 1) if repeat > 1 else contextlib.nullcontext()
            with loop:
                for p in range(NP):
                    G = g_fixed if compute_only else g_pool.tile(
                        [128, NCH * gb], gdt, tag="G")
                    if not compute_only:
                        for c in range(NCH):
                            seg = G[:, c * gb : (c + 1) * gb]
                            ioff = (p * NCH + c) * gidx_cols
                            if skip:
                                cnt = cnt_regs[c]
                                nc.gpsimd.reg_load(
                                    cnt,
                                    cnt_all[0:1, p * NCH + c : p * NCH + c + 1])
                            else:
                                cnt = gb
                            nc.gpsimd.dma_gather(
                                out_ap=seg.rearrange("p (j d) -> p j d", d=128),
                                in_ap=embeds[c * CH : (c + 1) * CH, :],
                                idxs_ap=idx_all[:, ioff : ioff + gidx_cols],
                                num_idxs=gb,
                                num_idxs_reg=cnt,
                                elem_size=D,
                                single_packet=single_packet,
                                queue_num=0 if one_queue else c,
                            )
                    if gather_only:
                        continue
                    for i in range(GRP):
                        w = p * GRP + i
                        acc = None if no_mm else psum_pool.tile(
                            [128, 128], mybir.dt.float32, space="PSUM")
                        k = 0
                        for c in range(NCH):
                            for j in range(nsub_ch):
                                scol = ((p * NCH + c) * GRP + i) * nsub_ch + j
                                if no_ts:
                                    S = s_fixed
                                elif act_mod and (
                                    (k % 5 in (2, 4)) if act_mod == 5
                                    else (k % 8 in (2, 5, 7)) if act_mod == 8
                                    else (k % 12 in (1, 3, 6, 8, 11))
                                    if act_mod == 12
                                    else k % act_mod == act_mod - 1
                                ):
                                    # ACT lane: T = |iota - row|;
                                    # S = relu(val - val*T) = val iff T==0
                                    T = t_pool.tile([128, 128],
                                                    mybir.dt.float32, tag="T")
                                    nc.scalar.activation(
                                        out=T[:], in_=iota_s[:],
                                        func=mybir.ActivationFunctionType.Abs,
                                        bias=nrows_all[:, scol : scol + 1],
                                    )
                                    S = sa_pool.tile([128, 128], gdt, tag="SA")
                                    nc.scalar.activation(
                                        out=S[:], in_=T[:],
                                        func=mybir.ActivationFunctionType.Relu,
                                        bias=vals_all[:, scol : scol + 1],
                                        scale=nvals_all[:, scol : scol + 1],
                                    )
                                else:
                                    S = s_pool.tile([128, SW], gdt, tag="S")
                                    nc.vector.tensor_scalar(
                                        out=S[:],
                                        in0=iota_f[:],
                                        scalar1=rows_all[:, scol : scol + 1],
                                        scalar2=vals_all[:, scol : scol + 1],
                                        op0=mybir.AluOpType.is_equal,
                                        op1=mybir.AluOpType.mult,
                                    )
                                gcol = (c * GRP + i) * budget + j * 128
                                if not no_mm:
                                    nc.tensor.matmul(
                                        out=acc[:],
                                        lhsT=S[:, 0:128],
                                        rhs=G[:, gcol : gcol + 128],
                                        start=(k == 0),
                                        stop=(k == nsub - 1),
                                    )
                                k += 1
                        if not no_mm:
                            o = o_pool.tile([128, 128], mybir.dt.float32)
                            nc.scalar.copy(out=o[:], in_=acc[:])
                            nc.sync.dma_start(
                                out=out[w * 128 : (w + 1) * 128, :], in_=o[:])

    nc.compile()
    return nc


def _balance(row_local, ch, core, limit=1024):
    """Assign each core's local rows to windows so per-(window, chunk) edge
    counts are near-uniform (greedy 4-dim vector bin packing, largest rows
    first, then swap-repair toward max <= limit). Returns (w, pos) per
    edge-row mapping arrays [N_CORES, B]."""
    w_of = np.empty((N_CORES, B), np.int64)
    pos_of = np.empty((N_CORES, B), np.int64)
    for k in range(N_CORES):
        m = core == k
        deg = np.zeros((B, NCH), np.int64)
        np.add.at(deg, (row_local[m], ch[m]), 1)
        order = np.argsort(-deg.sum(1), kind="stable")
        loads = np.zeros((NW, NCH), np.int64)
        cnt = np.zeros(NW, np.int64)
        wk = np.empty(B, np.int64)
        big = 1 << 50
        for r in order:
            cand = np.max(loads + deg[r], axis=1)
            cand[cnt >= WIN] = big
            wsel = int(np.argmin(cand))
            wk[r] = wsel
            loads[wsel] += deg[r]
            cnt[wsel] += 1
        # swap-repair: shave (window, chunk) cells above `limit`. Each accepted
        # swap leaves BOTH windows fully <= limit (feasibility is checked on
        # all 4 chunks), so over-limit cells strictly decrease -> terminates.
        stuck: set = set()
        for _ in range(800):
            over = np.argwhere(loads > limit)
            over = [t for t in map(tuple, over) if t not in stuck]
            if not over:
                break
            w1, c1 = max(over, key=lambda t: loads[t])
            rows_w1 = np.nonzero(wk == w1)[0]
            done = False
            for r1 in rows_w1[np.argsort(-deg[rows_w1, c1])][:32]:
                d1 = deg[r1]
                free_w = np.nonzero((cnt < WIN)
                                    & (np.arange(NW) != w1))[0]
                if len(free_w) and (loads[w1] - d1 <= limit).all():
                    okm = free_w[(loads[free_w] + d1 <= limit).all(1)]
                    if len(okm):
                        w2 = okm[np.argmin(loads[okm, c1])]
                        wk[r1] = w2
                        loads[w1] -= d1
                        loads[w2] += d1
                        cnt[w1] -= 1
                        cnt[w2] += 1
                        done = True
                        break
                for w2 in np.argsort(loads[:, c1]):
                    if w2 == w1:
                        continue
                    rows_w2 = np.nonzero(wk == w2)[0]
                    degs2 = deg[rows_w2]
                    ok = (((loads[w1] - d1)[None, :] + degs2 <= limit).all(1)
                          & ((loads[w2] + d1)[None, :] - degs2 <= limit).all(1))
                    idx2 = np.nonzero(ok)[0]
                    if len(idx2):
                        r2 = rows_w2[idx2[np.argmin(degs2[idx2, c1])]]
                        d2 = deg[r2]
                        wk[r1], wk[r2] = w2, w1
                        loads[w1] += d2 - d1
                        loads[w2] += d1 - d2
                        done = True
                        break
                if done:
                    break
            if not done:
                stuck.add((w1, c1))
        w_of[k] = wk
        for wsel in range(NW):
            rows_in = np.nonzero(wk == wsel)[0]
            pos_of[k, rows_in] = np.arange(len(rows_in))
    return w_of, pos_of


def _prep(edge_index, edge_vals, sort=None, skip=None, group=None,
          balance=None):
    """Bucket + sort + pad edges; returns (budget, per-core input dicts,
    per-core output row permutation out_perm [N_CORES, B] s.t.
    out[orig_row] = device_out[out_perm[orig_row]])."""
    if sort is None:
        sort = CFG["sort"]
    if skip is None:
        skip = CFG["skip"]
    if group is None:
        group = CFG["group"]
    if balance is None:
        balance = CFG.get("balance", True)
    GRP = group
    NP = NW // GRP

    rows = np.asarray(edge_index[0], dtype=np.int64)
    cols = np.asarray(edge_index[1], dtype=np.int64)
    vals = np.asarray(edge_vals, dtype=np.float32)
    E = rows.shape[0]

    core = rows // B
    row_local = rows - core * B
    ch = cols // CH
    col_rel = (cols - ch * CH).astype(np.int16)
    if balance:
        w_of, pos_of = _balance(row_local, ch, core)
        w = w_of[core, row_local]
        row_rel = pos_of[core, row_local].astype(np.float32)
        out_perm = w_of * WIN + pos_of          # [N_CORES, B]
    else:
        w = row_local // WIN
        row_rel = (row_local - w * WIN).astype(np.float32)
        out_perm = np.broadcast_to(np.arange(B, dtype=np.int64),
                                   (N_CORES, B))

    # slot-space order: (core, group, chunk, window-in-group, pos)
    p = w // GRP
    i = w - p * GRP
    bucket = (((core * NP + p) * NCH + ch) * GRP + i).astype(np.int64)
    n_buckets = N_CORES * NW * NCH
    counts = np.bincount(bucket, minlength=n_buckets)
    budget = int(-(-counts.max() // 128) * 128)

    # bucket-major, column-sorted within bucket: monotone HBM addresses
    order = np.lexsort((cols, bucket)) if sort else np.argsort(bucket, kind="stable")
    starts = np.zeros(n_buckets, dtype=np.int64)
    np.cumsum(counts[:-1], out=starts[1:])
    pos = np.arange(E, dtype=np.int64) - starts[bucket[order]]

    bo = bucket[order]
    slot = bo * budget + pos            # global slot id across all cores

    n_slots = n_buckets * budget
    # interior windows of a group pad with 0 (gathered, weight 0); the last
    # window's padding is -1 (descriptor-skipped) when skip is on
    if skip:
        idx_lin = np.zeros(n_slots, dtype=np.int16)
        tail = (np.arange(n_buckets, dtype=np.int64) % GRP) == GRP - 1
        idx_lin[np.repeat(tail, budget)] = -1
    else:
        idx_lin = np.zeros(n_slots, dtype=np.int16)
    rows_lin = np.zeros(n_slots, dtype=np.float32)
    vals_lin = np.zeros(n_slots, dtype=np.float32)
    idx_lin[slot] = col_rel[order]
    rows_lin[slot] = row_rel[order]
    vals_lin[slot] = vals[order]

    # per-(group, chunk) gather count: interior windows fully gathered, the
    # last window only its real edges
    cg = counts.reshape(-1, GRP)  # [cores*NP*NCH, GRP]
    call_counts = (GRP - 1) * budget + cg[:, GRP - 1]
    if skip and GRP == 1:
        # an all-negative index list is invalid; give empty buckets one
        # dummy slot (gather row 0, weight 0)
        empty = call_counts == 0
        if empty.any():
            eb = np.nonzero(empty)[0]
            idx_lin[eb * budget] = 0
        call_counts = np.maximum(call_counts, 1)
    call_counts = call_counts.astype(np.int32)

    in_maps = []
    per_core = NW * NCH * budget
    n_calls = NP * NCH
    for k in range(N_CORES):
        lin = slice(k * per_core, (k + 1) * per_core)
        # idx16: slot q of a call -> (partition q%16, col q//16), tiled x8
        a = idx_lin[lin].reshape(n_calls, GRP * budget // 16, 16)
        idx16 = np.ascontiguousarray(a.transpose(2, 0, 1).reshape(16, -1))
        idx16 = np.tile(idx16, (8, 1))
        # rows/vals: slot s -> (partition s%128, col s//128)
        rows_t = np.ascontiguousarray(rows_lin[lin].reshape(-1, 128).T)
        vals_t = np.ascontiguousarray(vals_lin[lin].reshape(-1, 128).T)
        m = {"idx16": idx16, "rows_rel": rows_t, "vals": vals_t,
             "neg_rows": -rows_t, "neg_vals": -vals_t}
        if skip:
            m["counts"] = call_counts[k * n_calls : (k + 1) * n_calls].reshape(
                1, -1)
        in_maps.append(m)
    return budget, in_maps, out_perm


def kernel(embeds, edge_index, edge_vals):
    from concourse.bass_utils import run_bass_kernel_spmd

    embeds = np.ascontiguousarray(np.asarray(embeds, dtype=np.float32))
    if CFG["bf16"]:
        embeds = embeds.astype(BF16)
    budget, in_maps, out_perm = _prep(edge_index, edge_vals)
    for m in in_maps:
        m["embeds"] = embeds

    if budget not in _cache:
        _cache[budget] = _build(budget)
    nc = _cache[budget]

    res = run_bass_kernel_spmd(nc, in_maps, core_ids=list(range(N_CORES)))
    out = np.empty((N_NODES, D), dtype=np.float32)
    for k in range(N_CORES):
        out[k * B : (k + 1) * B] = res.results[k]["out"][out_perm[k]]
    return out



# revision 28
# speedup vs baseline: 1.0498x; 1.0043x over previous
"""GCN layer (SpMM): out[r] = sum_{e: row(e)=r} val[e] * embeds[col(e)]
for N=100000 nodes, d=128, E=3200000 edges, distributed over 8 NeuronCores.

Sharding: 1D row partition — core k owns destination rows [k*12500, (k+1)*12500);
the embedding table is replicated. Per core the edges are bucketed by
(128-row output window, 25000-row column chunk); each bucket is padded to a
common slot budget so one SPMD program serves all cores.

Key design points (see git history for the f32 baseline this evolved from):
  - embeds cast to bf16 on host: fp32 matmuls run at 1/4 rate on the PE
    (2 half-speed passes), bf16 at full rate; and each gather descriptor
    moves 256B instead of 512B.
  - buckets are sorted by column index so the random HBM reads of one
    bucket walk addresses monotonically (row-buffer locality).
  - buckets are padded with idx=-1 and the true count is passed through
    num_idxs_reg (loaded from SBUF at runtime), so the SWDGE skips the
    padding descriptors entirely instead of gathering dummy rows.
  - `group` consecutive windows share one dma_gather per chunk, amortizing
    the ~1us fixed SWDGE cost per call; interior windows pad with idx 0
    (gathered but weighted 0), only the last window's padding is skipped.

Device pipeline per window group:
  - NCH dma_gathers (one per column chunk, int16 chunk-relative indices)
    pull the 256B bf16 embedding rows for the group's edges into SBUF.
  - per 128-edge subtile, one fused DVE tensor_scalar builds the
    val-weighted one-hot S[e, r] = val[e] * (row_rel[e] == r).
  - TensorE matmuls S^T @ G accumulate each window's [128,128] block in
    PSUM; PSUM -> SBUF -> DRAM.
"""

import os
import sys

import numpy as np

for _p in ("/opt/trn_rl_repo", "/root/problem"):
    if _p not in sys.path:
        sys.path.insert(0, _p)

os.environ.setdefault("NEURON_RT_RESET_CORES", "1")

import ml_dtypes

BF16 = np.dtype(ml_dtypes.bfloat16)

N_NODES = 100000
D = 128
N_CORES = 8
B = N_NODES // N_CORES          # 12500 destination rows per core
WIN = 128                       # output window rows (= PSUM partition dim)
NW = 100                        # windows per core (2 windows of slack so the
                                # balancer can hold every bucket <= 1024)
B_PAD = NW * WIN                # 12800 padded rows per core
NCH = 4                         # column chunks (int16 index range)
CH = N_NODES // NCH             # 25000 rows per chunk

_cache = {}

# compute-path dtype, bucket sorting, padding-skip, and gather grouping are
# switchable so the perf harness can A/B them; defaults ship.
CFG = dict(bf16=True, sort=True, skip=True, group=2, psum_iota=True,
           act_mod=12, iota_mode=None, balance=True)


def _build(budget, repeat=1, bf16=None, skip=None, group=None,
           psum_iota=None, single_packet=False, one_queue=False,
           dma_scratch=16384, act_mod=None, gather_only=False,
           compute_only=False, no_ts=False, no_mm=False, iota_mode=None,
           g_bufs=None):
    """Build + schedule the SPMD bass program for a per-(window,chunk) slot
    budget (multiple of 128). Returns the compiled Bacc module.

    repeat > 1 wraps the compute body in an on-device For_i loop — used only
    by the perf harness to amortize dispatch overhead when measuring."""
    import contextlib

    import concourse.mybir as mybir
    import concourse.tile as tile
    from concourse import bacc

    if bf16 is None:
        bf16 = CFG["bf16"]
    if skip is None:
        skip = CFG["skip"]
    if group is None:
        group = CFG["group"]
    if psum_iota is None:
        psum_iota = CFG["psum_iota"]
    if act_mod is None:
        act_mod = CFG["act_mod"]
    if iota_mode is None:
        iota_mode = CFG.get("iota_mode")
    # iota_mode: None -> legacy f32 iota (PSUM or SBUF per psum_iota);
    # 'bf16_4x' -> bf16 SBUF iota: DVE picks a 2-port packed mode, which
    # locks GPSIMD out of the shared SBUF port pair and starves SWDGE
    # descriptor gen (measured: gathers fully serialize against S-builds);
    # 'bf16_2x' -> bf16 SBUF iota, [128,129] S (odd innermost) — measured
    # identical to bf16_4x, both blocking;
    # 'psum_bf16' -> bf16 iota in its own PSUM bank: in0 arrives via the
    # PSUM port, per-partition scalars via the dedicated rd0, S via the
    # dedicated write port -> 2x_1P at 120+FD/2 cycles with ZERO shared-pair
    # usage. Non-blocking and 1.35x faster than the legacy f32 PSUM read.
    SW = 129 if iota_mode == "bf16_2x" else 128
    GRP = group
    NP = NW // GRP
    assert NP * GRP == NW

    nsub_ch = budget // 128          # subtiles per (window, chunk) segment
    nsub = NCH * nsub_ch             # subtiles per window
    gb = GRP * budget                # slots per gather call
    gidx_cols = gb // 16             # idx16 columns per (group, chunk)
    if g_bufs is None:
        g_bufs = 6 if GRP == 1 else (3 if GRP == 2 else 2)

    nc = bacc.Bacc("TRN2", target_bir_lowering=False, debug=False,
                   num_devices=N_CORES, num_swdge_queues=4,
                   dynamic_dma_scratch_size=dma_scratch)
    gdt = mybir.dt.bfloat16 if bf16 else mybir.dt.float32
    embeds = nc.dram_tensor("embeds", [N_NODES, D], gdt,
                            kind="ExternalInput")
    idx16 = nc.dram_tensor("idx16", [128, NP * NCH * gidx_cols],
                           mybir.dt.int16, kind="ExternalInput")
    rows_rel = nc.dram_tensor("rows_rel", [128, NW * nsub], mybir.dt.float32,
                              kind="ExternalInput")
    vals = nc.dram_tensor("vals", [128, NW * nsub], mybir.dt.float32,
                          kind="ExternalInput")
    neg_rows = nc.dram_tensor("neg_rows", [128, NW * nsub], mybir.dt.float32,
                              kind="ExternalInput")
    neg_vals = nc.dram_tensor("neg_vals", [128, NW * nsub], mybir.dt.float32,
                              kind="ExternalInput")
    counts = nc.dram_tensor("counts", [1, NP * NCH], mybir.dt.int32,
                            kind="ExternalInput") if skip else None
    out = nc.dram_tensor("out", [B_PAD, D], mybir.dt.float32,
                         kind="ExternalOutput")

    with tile.TileContext(nc) as tc:
        with (
            tc.tile_pool(name="const", bufs=1) as const_pool,
            tc.tile_pool(name="gather", bufs=g_bufs) as g_pool,
            tc.tile_pool(name="s", bufs=12) as s_pool,
            tc.tile_pool(name="sa", bufs=6) as sa_pool,
            tc.tile_pool(name="t", bufs=6) as t_pool,
            tc.tile_pool(name="o", bufs=4) as o_pool,
            tc.tile_pool(name="psum", bufs=7, space="PSUM") as psum_pool,
            tc.tile_pool(name="iotap", bufs=1, space="PSUM") as iota_pool,
        ):
            iota_i = const_pool.tile([128, SW], mybir.dt.int32)
            nc.gpsimd.iota(iota_i[:], pattern=[[1, SW]], base=0,
                           channel_multiplier=0)
            if iota_mode == "psum_bf16":
                # bf16 iota in its own full PSUM bank (1024 bf16 = 2KB per
                # partition) so it never shares a bank with a PE accumulator.
                iota_bank = iota_pool.tile([128, 1024], mybir.dt.bfloat16,
                                           space="PSUM")
                iota_f = iota_bank[:, 0:128]
            elif iota_mode is not None:
                # bf16 SBUF iota: integer values 0..128 are exact in bf16 and
                # unlock the DVE 16-bit packed perf modes for the S-build.
                iota_f = const_pool.tile([128, SW], mybir.dt.bfloat16)
            elif psum_iota:
                # legacy: iota lives in PSUM so the f32 tensor_scalar stays in
                # 1x perf mode and never takes the shared SBUF port pair that
                # SWDGE descriptor generation (gpsimd) needs. Pad to a full
                # 2KB-per-partition PSUM bank so the iota never shares a bank
                # with a PE-written accumulator.
                iota_bank = iota_pool.tile([128, 512], mybir.dt.float32,
                                           space="PSUM")
                iota_f = iota_bank[:, 0:128]
            else:
                iota_f = const_pool.tile([128, 128], mybir.dt.float32)
            nc.vector.tensor_copy(out=iota_f[:], in_=iota_i[:])

            idx_all = const_pool.tile([128, NP * NCH * gidx_cols],
                                      mybir.dt.int16)
            nc.sync.dma_start(out=idx_all[:], in_=idx16[:])
            rows_all = const_pool.tile([128, NW * nsub], mybir.dt.float32)
            nc.sync.dma_start(out=rows_all[:], in_=rows_rel[:])
            vals_all = const_pool.tile([128, NW * nsub], mybir.dt.float32)
            nc.sync.dma_start(out=vals_all[:], in_=vals[:])
            nrows_all = const_pool.tile([128, NW * nsub], mybir.dt.float32)
            nc.sync.dma_start(out=nrows_all[:], in_=neg_rows[:])
            nvals_all = const_pool.tile([128, NW * nsub], mybir.dt.float32)
            nc.sync.dma_start(out=nvals_all[:], in_=neg_vals[:])
            # ACT-side S-build needs a plain SBUF f32 iota even when the DVE
            # side reads its iota from PSUM or as bf16
            # bf16 iota + bf16 T keep both ACT ops on 16-bit packed operands
            # (ScalarE 2x mode: 222+FD/2 vs 222+FD cycles); |i - row| <= 127
            # is exact in bf16 so numerics are unchanged.
            iota_s = const_pool.tile([128, 128], mybir.dt.bfloat16)
            nc.vector.tensor_copy(out=iota_s[:], in_=iota_i[:, 0:128])
            if skip:
                cnt_all = const_pool.tile([1, NP * NCH], mybir.dt.int32)
                nc.sync.dma_start(out=cnt_all[:], in_=counts[:])
                # Pre-zero the gather ring so slots the SWDGE skips (trailing
                # -1 indices) never expose NaN garbage to the 0*x matmul
                # terms.
                if not compute_only:
                    for _ in range(g_bufs):
                        gz = g_pool.tile([128, NCH * gb], gdt, tag="G")
                        nc.vector.memset(gz[:], 0)
                cnt_regs = [
                    nc.gpsimd.alloc_register(f"cnt{c}") for c in range(NCH)
                ]
            if compute_only:
                g_fixed = const_pool.tile([128, NCH * gb], gdt)
                nc.vector.memset(g_fixed[:], 0)
            if no_ts:
                s_fixed = const_pool.tile([128, 128], gdt)
                nc.vector.memset(s_fixed[:], 0)

            loop = tc.For_i(0, repeat, 1) if repeat > 1 else contextlib.nullcontext()
            with loop:
                for p in range(NP):
                    G = g_fixed if compute_only else g_pool.tile(
                        [128, NCH * gb], gdt, tag="G")
                    if not compute_only:
                        for c in range(NCH):
                            seg = G[:, c * gb : (c + 1) * gb]
                            ioff = (p * NCH + c) * gidx_cols
                            if skip:
                                cnt = cnt_regs[c]
                                nc.gpsimd.reg_load(
                                    cnt,
                                    cnt_all[0:1, p * NCH + c : p * NCH + c + 1])
                            else:
                                cnt = gb
                            nc.gpsimd.dma_gather(
                                out_ap=seg.rearrange("p (j d) -> p j d", d=128),
                                in_ap=embeds[c * CH : (c + 1) * CH, :],
                                idxs_ap=idx_all[:, ioff : ioff + gidx_cols],
                                num_idxs=gb,
                                num_idxs_reg=cnt,
                                elem_size=D,
                                single_packet=single_packet,
                                queue_num=0 if one_queue else c,
                            )
                    if gather_only:
                        continue
                    for i in range(GRP):
                        w = p * GRP + i
                        acc = None if no_mm else psum_pool.tile(
                            [128, 128], mybir.dt.float32, space="PSUM")
                        k = 0
                        for c in range(NCH):
                            for j in range(nsub_ch):
                                scol = ((p * NCH + c) * GRP + i) * nsub_ch + j
                                if no_ts:
                                    S = s_fixed
                                elif act_mod and (
                                    (k % 5 in (2, 4)) if act_mod == 5
                                    else (k % 8 in (2, 5, 7)) if act_mod == 8
                                    else (k % 12 in (1, 3, 6, 8, 11))
                                    if act_mod == 12
                                    else k % act_mod == act_mod - 1
                                ):
                                    # ACT lane: T = |iota - row|;
                                    # S = relu(val - val*T) = val iff T==0
                                    T = t_pool.tile([128, 128],
                                                    mybir.dt.bfloat16, tag="T")
                                    nc.scalar.activation(
                                        out=T[:], in_=iota_s[:],
                                        func=mybir.ActivationFunctionType.Abs,
                                        bias=nrows_all[:, scol : scol + 1],
                                    )
                                    S = sa_pool.tile([128, 128], gdt, tag="SA")
                                    nc.scalar.activation(
                                        out=S[:], in_=T[:],
                                        func=mybir.ActivationFunctionType.Relu,
                                        bias=vals_all[:, scol : scol + 1],
                                        scale=nvals_all[:, scol : scol + 1],
                                    )
                                else:
                                    S = s_pool.tile([128, SW], gdt, tag="S")
                                    nc.vector.tensor_scalar(
                                        out=S[:],
                                        in0=iota_f[:],
                                        scalar1=rows_all[:, scol : scol + 1],
                                        scalar2=vals_all[:, scol : scol + 1],
                                        op0=mybir.AluOpType.is_equal,
                                        op1=mybir.AluOpType.mult,
                                    )
                                gcol = (c * GRP + i) * budget + j * 128
                                if not no_mm:
                                    nc.tensor.matmul(
                                        out=acc[:],
                                        lhsT=S[:, 0:128],
                                        rhs=G[:, gcol : gcol + 128],
                                        start=(k == 0),
                                        stop=(k == nsub - 1),
                                    )
                                k += 1
                        if not no_mm:
                            o = o_pool.tile([128, 128], mybir.dt.float32)
                            nc.scalar.copy(out=o[:], in_=acc[:])
                            nc.sync.dma_start(
                                out=out[w * 128 : (w + 1) * 128, :], in_=o[:])

    nc.compile()
    return nc


def _balance(row_local, ch, core, limit=1024):
    """Assign each core's local rows to windows so per-(window, chunk) edge
    counts are near-uniform (greedy 4-dim vector bin packing, largest rows
    first, then swap-repair toward max <= limit). Returns (w, pos) per
    edge-row mapping arrays [N_CORES, B]."""
    w_of = np.empty((N_CORES, B), np.int64)
    pos_of = np.empty((N_CORES, B), np.int64)
    for k in range(N_CORES):
        m = core == k
        deg = np.zeros((B, NCH), np.int64)
        np.add.at(deg, (row_local[m], ch[m]), 1)
        order = np.argsort(-deg.sum(1), kind="stable")
        loads = np.zeros((NW, NCH), np.int64)
        cnt = np.zeros(NW, np.int64)
        wk = np.empty(B, np.int64)
        big = 1 << 50
        for r in order:
            cand = np.max(loads + deg[r], axis=1)
            cand[cnt >= WIN] = big
            wsel = int(np.argmin(cand))
            wk[r] = wsel
            loads[wsel] += deg[r]
            cnt[wsel] += 1
        # swap-repair: shave (window, chunk) cells above `limit`. Each accepted
        # swap leaves BOTH windows fully <= limit (feasibility is checked on
        # all 4 chunks), so over-limit cells strictly decrease -> terminates.
        stuck: set = set()
        for _ in range(800):
            over = np.argwhere(loads > limit)
            over = [t for t in map(tuple, over) if t not in stuck]
            if not over:
                break
            w1, c1 = max(over, key=lambda t: loads[t])
            rows_w1 = np.nonzero(wk == w1)[0]
            done = False
            for r1 in rows_w1[np.argsort(-deg[rows_w1, c1])][:32]:
                d1 = deg[r1]
                free_w = np.nonzero((cnt < WIN)
                                    & (np.arange(NW) != w1))[0]
                if len(free_w) and (loads[w1] - d1 <= limit).all():
                    okm = free_w[(loads[free_w] + d1 <= limit).all(1)]
                    if len(okm):
                        w2 = okm[np.argmin(loads[okm, c1])]
                        wk[r1] = w2
                        loads[w1] -= d1
                        loads[w2] += d1
                        cnt[w1] -= 1
                        cnt[w2] += 1
                        done = True
                        break
                for w2 in np.argsort(loads[:, c1]):
                    if w2 == w1:
                        continue
                    rows_w2 = np.nonzero(wk == w2)[0]
                    degs2 = deg[rows_w2]
                    ok = (((loads[w1] - d1)[None, :] + degs2 <= limit).all(1)
                          & ((loads[w2] + d1)[None, :] - degs2 <= limit).all(1))
                    idx2 = np.nonzero(ok)[0]
                    if len(idx2):
                        r2 = rows_w2[idx2[np.argmin(degs2[idx2, c1])]]
                        d2 = deg[r2]
                        wk[r1], wk[r2] = w2, w1
                        loads[w1] += d2 - d1
                        loads[w2] += d1 - d2
                        done = True
                        break
                if done:
                    break
            if not done:
                stuck.add((w1, c1))
        w_of[k] = wk
        for wsel in range(NW):
            rows_in = np.nonzero(wk == wsel)[0]
            pos_of[k, rows_in] = np.arange(len(rows_in))
    return w_of, pos_of


def _prep(edge_index, edge_vals, sort=None, skip=None, group=None,
          balance=None):
    """Bucket + sort + pad edges; returns (budget, per-core input dicts,
    per-core output row permutation out_perm [N_CORES, B] s.t.
    out[orig_row] = device_out[out_perm[orig_row]])."""
    if sort is None:
        sort = CFG["sort"]
    if skip is None:
        skip = CFG["skip"]
    if group is None:
        group = CFG["group"]
    if balance is None:
        balance = CFG.get("balance", True)
    GRP = group
    NP = NW // GRP

    rows = np.asarray(edge_index[0], dtype=np.int64)
    cols = np.asarray(edge_index[1], dtype=np.int64)
    vals = np.asarray(edge_vals, dtype=np.float32)
    E = rows.shape[0]

    core = rows // B
    row_local = rows - core * B
    ch = cols // CH
    col_rel = (cols - ch * CH).astype(np.int16)
    if balance:
        w_of, pos_of = _balance(row_local, ch, core)
        w = w_of[core, row_local]
        row_rel = pos_of[core, row_local].astype(np.float32)
        out_perm = w_of * WIN + pos_of          # [N_CORES, B]
    else:
        w = row_local // WIN
        row_rel = (row_local - w * WIN).astype(np.float32)
        out_perm = np.broadcast_to(np.arange(B, dtype=np.int64),
                                   (N_CORES, B))

    # slot-space order: (core, group, chunk, window-in-group, pos)
    p = w // GRP
    i = w - p * GRP
    bucket = (((core * NP + p) * NCH + ch) * GRP + i).astype(np.int64)
    n_buckets = N_CORES * NW * NCH
    counts = np.bincount(bucket, minlength=n_buckets)
    budget = int(-(-counts.max() // 128) * 128)

    # bucket-major, column-sorted within bucket: monotone HBM addresses
    order = np.lexsort((cols, bucket)) if sort else np.argsort(bucket, kind="stable")
    starts = np.zeros(n_buckets, dtype=np.int64)
    np.cumsum(counts[:-1], out=starts[1:])
    pos = np.arange(E, dtype=np.int64) - starts[bucket[order]]

    bo = bucket[order]
    slot = bo * budget + pos            # global slot id across all cores

    n_slots = n_buckets * budget
    # interior windows of a group pad with 0 (gathered, weight 0); the last
    # window's padding is -1 (descriptor-skipped) when skip is on
    if skip:
        idx_lin = np.zeros(n_slots, dtype=np.int16)
        tail = (np.arange(n_buckets, dtype=np.int64) % GRP) == GRP - 1
        idx_lin[np.repeat(tail, budget)] = -1
    else:
        idx_lin = np.zeros(n_slots, dtype=np.int16)
    rows_lin = np.zeros(n_slots, dtype=np.float32)
    vals_lin = np.zeros(n_slots, dtype=np.float32)
    idx_lin[slot] = col_rel[order]
    rows_lin[slot] = row_rel[order]
    vals_lin[slot] = vals[order]

    # per-(group, chunk) gather count: interior windows fully gathered, the
    # last window only its real edges
    cg = counts.reshape(-1, GRP)  # [cores*NP*NCH, GRP]
    call_counts = (GRP - 1) * budget + cg[:, GRP - 1]
    if skip and GRP == 1:
        # an all-negative index list is invalid; give empty buckets one
        # dummy slot (gather row 0, weight 0)
        empty = call_counts == 0
        if empty.any():
            eb = np.nonzero(empty)[0]
            idx_lin[eb * budget] = 0
        call_counts = np.maximum(call_counts, 1)
    call_counts = call_counts.astype(np.int32)

    in_maps = []
    per_core = NW * NCH * budget
    n_calls = NP * NCH
    for k in range(N_CORES):
        lin = slice(k * per_core, (k + 1) * per_core)
        # idx16: slot q of a call -> (partition q%16, col q//16), tiled x8
        a = idx_lin[lin].reshape(n_calls, GRP * budget // 16, 16)
        idx16 = np.ascontiguousarray(a.transpose(2, 0, 1).reshape(16, -1))
        idx16 = np.tile(idx16, (8, 1))
        # rows/vals: slot s -> (partition s%128, col s//128)
        rows_t = np.ascontiguousarray(rows_lin[lin].reshape(-1, 128).T)
        vals_t = np.ascontiguousarray(vals_lin[lin].reshape(-1, 128).T)
        m = {"idx16": idx16, "rows_rel": rows_t, "vals": vals_t,
             "neg_rows": -rows_t, "neg_vals": -vals_t}
        if skip:
            m["counts"] = call_counts[k * n_calls : (k + 1) * n_calls].reshape(
                1, -1)
        in_maps.append(m)
    return budget, in_maps, out_perm


def kernel(embeds, edge_index, edge_vals):
    from concourse.bass_utils import run_bass_kernel_spmd

    embeds = np.ascontiguousarray(np.asarray(embeds, dtype=np.float32))
    if CFG["bf16"]:
        embeds = embeds.astype(BF16)
    budget, in_maps, out_perm = _prep(edge_index, edge_vals)
    for m in in_maps:
        m["embeds"] = embeds

    if budget not in _cache:
        _cache[budget] = _build(budget)
    nc = _cache[budget]

    res = run_bass_kernel_spmd(nc, in_maps, core_ids=list(range(N_CORES)))
    out = np.empty((N_NODES, D), dtype=np.float32)
    for k in range(N_CORES):
        out[k * B : (k + 1) * B] = res.results[k]["out"][out_perm[k]]
    return out

